# revision 60
# baseline (speedup 1.0000x reference)
"""AxialAttention3D Trainium2 Bass kernel (fp8 DoubleRow + slice pipelining).

Reference computes, for x [B=2, C=512, D=32, H=32, W=32]:
  qkv = 1x1x1 conv (w_qkv [1536,512]) -> q,k,v [B,512,D,H,W]
  8-head attention along the D axis, independent per (b,h,w,head), hd=64
  out = 1x1x1 conv (w_out) + b_out + x  (residual)

Sharding: 64 (b,h)-slices split across 8 cores (8 slices/core). Each slice is
x[b,:,:,h,:] = [C=512, N=1024 tokens] with tokens in w-major (w,d) order
(host pre-permutes, so all device access is contiguous).

Precision: the three projections (97% of FLOPs) run in fp8e4 with
MatmulPerfMode.DoubleRow (two 128-deep K planes per instruction; measured 2x
bf16 throughput). Weights are scaled x16 on host so their values sit in
fp8e4's normal range; the PSUM->SBUF copy unscales by 1/16. Attention
(scores/softmax/AV) stays bf16; ao is stored fp8 as the out-proj moving
operand. Residual: host sends xres = 16*(x + bout_eff) bf16; device adds PSUM
(16*proj) and stores bf16; host divides by 16. b_v commutes through softmax
(rows sum to 1) and is folded into bout_eff on host.

Schedule (per core): attention runs on PAIRS of w-groups (4 pair-iterations +
1 drain per slice) so one softmax chain (exp on Act -> reduce/recip on DVE ->
normalize-mul on Pool -> 32x32 block transpose on DVE, ~4us) is amortized over
~7.5us of PE work. Projections for slice s+1 are interleaved into slice s's
attention loop: QK chunks front-loaded (pair-iters 0-2), V^T chunks late
(pair-iters 3-4), out-projection per token half as soon as its two pairs are
copied. The PE is never parked on the softmax latency chain except at the
first/last slice boundaries.

PSUM (8 banks): projections 2x[128,512]f32; scores one [128,2(par),512]f32
tile whose dim1 stride is a 2KB bank (concurrent quadrant matmuls sharing a
PE column-group must accumulate in different banks - par0/par1 write the same
column group); AV one [128,4(wq),512]f32 tile banked by w-row-group.
Successive pairs alternate the 256-column halves of the score/AV banks, so
scores(k+1) never serializes behind exp(k) nor AV(k+1) behind the AV copy(k).

Cross-slice anti-dependency slack: qk/vt pools are triple-buffered and all
QK chunk schedules are token-half-major (n=0 halves first), since slice s+1's
first two attention pairs read only the n=0 half of qk(s+1) -- region
tracking lets them start while the n=1 half is still in flight.

Engine budget at ~260us: Tensor ~207us busy (proj 512 DR matmuls ~110us,
attention 4096 32-col matmuls ~62us + ldweights), Act ~143us (exp + qk/vt
copies), DVE ~143us (AV copies, residual adds, reduce/recip/transpose),
Pool ~73us (softmax normalize) + output-store DMA ring.
"""

import os
import sys

import numpy as np
import ml_dtypes

sys.path.insert(0, "/opt/trn_rl_repo")

B, C, D, H, W = 2, 512, 32, 32, 32
NH, HD = 8, 64
NCORES = 8
S = (B * H) // NCORES  # 8 slices per core
NTOK = D * W  # 1024 tokens per slice
WS = 16.0  # fp8 weight prescale

LAST_RESULTS = None  # set on each kernel() call; test harness reads exec time


def _build():
    import concourse.bass as bass  # noqa: F401
    from concourse import bacc, mybir
    import concourse.tile as tile

    bf16 = mybir.dt.bfloat16
    f32 = mybir.dt.float32
    f8 = mybir.dt.float8e4
    Act = mybir.ActivationFunctionType
    DR = mybir.MatmulPerfMode.DoubleRow

    nc = bacc.Bacc("TRN2", target_bir_lowering=False, debug=False)

    xs8_d = nc.dram_tensor("xs8", [S, C, NTOK], f8, kind="ExternalInput")
    xres_d = nc.dram_tensor("xres", [S, C, NTOK], bf16, kind="ExternalInput")
    wqkT_d = nc.dram_tensor("wqkT", [C, 2 * C], f8, kind="ExternalInput")
    wvT_d = nc.dram_tensor("wvT", [C, C], f8, kind="ExternalInput")
    woutT_d = nc.dram_tensor("woutT", [C, C], f8, kind="ExternalInput")
    bqk_d = nc.dram_tensor("bqk", [2 * C], f32, kind="ExternalInput")
    out_d = nc.dram_tensor("out", [S, C, NTOK], bf16, kind="ExternalOutput")

    with tile.TileContext(nc) as tc:
        with tc.tile_pool(name="consts", bufs=1) as consts, \
             tc.tile_pool(name="x8p", bufs=3) as x8p, \
             tc.tile_pool(name="xrp", bufs=3) as xrp, \
             tc.tile_pool(name="qkp", bufs=3) as qkp, \
             tc.tile_pool(name="vtp", bufs=3) as vtp, \
             tc.tile_pool(name="aop", bufs=3) as aop, \
             tc.tile_pool(name="pp", bufs=4) as pp, \
             tc.tile_pool(name="ttp", bufs=4) as ttp, \
             tc.tile_pool(name="smp", bufs=3) as smp, \
             tc.tile_pool(name="outp", bufs=8) as outp, \
             tc.tile_pool(name="psmm", bufs=2, space="PSUM") as psmm, \
             tc.tile_pool(name="pss", bufs=1, space="PSUM") as pss, \
             tc.tile_pool(name="psav", bufs=1, space="PSUM") as psav:

            x8t, xrt, qkt, vtt = {}, {}, {}, {}

            def prefetch(s):
                if s >= S or s in x8t:
                    return
                x8 = x8p.tile([128, 4, NTOK], f8, tag="x8", name=f"x8_{s}")
                xr = xrp.tile([128, 4, NTOK], bf16, tag="xr", name=f"xr_{s}")
                if s == 0:
                    # split by token half so A(0)'s (t,0) chunks start after
                    # the first half lands (region-tracked partial overlap)
                    for h in range(2):
                        nc.sync.dma_start(
                            out=x8[:, :, h * 512:(h + 1) * 512],
                            in_=xs8_d.ap()[s, :, h * 512:(h + 1) * 512].rearrange(
                                "(k p) t -> p k t", p=128))
                else:
                    nc.sync.dma_start(
                        out=x8, in_=xs8_d.ap()[s].rearrange("(k p) t -> p k t", p=128))
                # slice 0's residual rides the (empty-at-startup) gpsimd ring
                # so the first QK matmul doesn't queue behind 1MB of xres
                xr_eng = nc.gpsimd if s == 0 else nc.sync
                xr_eng.dma_start(
                    out=xr, in_=xres_d.ap()[s].rearrange("(k p) t -> p k t", p=128))
                x8t[s], xrt[s] = x8, xr

            # ---- constants (x8(0) and wqkT first: A(0) QK needs only these) ----
            prefetch(0)
            bqk_sb = consts.tile([128, 8], f32)  # [o%128, o//128]
            wqkT_a = consts.tile([128, 4, C], f8)  # [c'%128, c'//128, o] o<512
            wqkT_b = consts.tile([128, 4, C], f8)  # o>=512
            wvT_sb = consts.tile([128, 4, C], f8)
            woutT_sb = consts.tile([128, 4, C], f8)
            nc.sync.dma_start(
                out=wqkT_a, in_=wqkT_d.ap()[:, 0:C].rearrange("(k p) o -> p k o", p=128))
            nc.sync.dma_start(out=bqk_sb, in_=bqk_d.ap().rearrange("(t p) -> p t", p=128))
            nc.sync.dma_start(
                out=wqkT_b, in_=wqkT_d.ap()[:, C:2 * C].rearrange("(k p) o -> p k o", p=128))
            nc.sync.dma_start(
                out=wvT_sb, in_=wvT_d.ap().rearrange("(k p) o -> p k o", p=128))
            nc.sync.dma_start(
                out=woutT_sb, in_=woutT_d.ap().rearrange("(k p) o -> p k o", p=128))
            

            def a_alloc(s):
                qkt[s] = qkp.tile([128, 8, NTOK], bf16, tag="qk", name=f"qk_{s}")
                vtt[s] = vtp.tile([128, 8, C], bf16, tag="vt", name=f"vt_{s}")

            def a_qk_half(s, t, n, dve=False):
                # QK projection, output tile t (128 of 1024 q|k chans), token half n
                x8, qk = x8t[s], qkt[s]
                ps = psmm.tile([128, 512], f32, tag="proj", name="ps_qk")
                wsb = wqkT_a if t < 4 else wqkT_b
                for j in range(2):
                    nc.tensor.matmul(
                        ps,
                        wsb[:, 2 * j:2 * j + 2, (t % 4) * 128:(t % 4 + 1) * 128],
                        x8[:, 2 * j:2 * j + 2, n * 512:(n + 1) * 512],
                        start=(j == 0), stop=(j == 1), perf_mode=DR)
                out = qk[:, t, n * 512:(n + 1) * 512]
                if dve:
                    nc.vector.tensor_scalar(
                        out=out, in0=ps, scalar1=1.0 / WS, scalar2=bqk_sb[:, t:t + 1],
                        op0=mybir.AluOpType.mult, op1=mybir.AluOpType.add)
                else:
                    nc.scalar.activation(
                        out=out, in_=ps,
                        func=Act.Identity, bias=bqk_sb[:, t:t + 1], scale=1.0 / WS)

            def a_vt_chunk(s, g, dve=False):
                # V^T projection for token block g (tokens on partitions)
                x8, vt = x8t[s], vtt[s]
                ps = psmm.tile([128, 512], f32, tag="proj", name="ps_vt")
                for j in range(2):
                    nc.tensor.matmul(
                        ps,
                        x8[:, 2 * j:2 * j + 2, g * 128:(g + 1) * 128],
                        wvT_sb[:, 2 * j:2 * j + 2, :],
                        start=(j == 0), stop=(j == 1), perf_mode=DR)
                if dve:
                    nc.vector.tensor_scalar_mul(vt[:, g, :], ps, 1.0 / WS)
                else:
                    nc.scalar.activation(out=vt[:, g, :], in_=ps,
                                         func=Act.Copy, scale=1.0 / WS)

            def out_half(s, n, ao, t0=0, t1=4):
                # out projection + residual for token half n, out tiles [t0,t1)
                xr = xrt[s]
                for t in range(t0, t1):
                    ps = psmm.tile([128, 512], f32, tag="proj", name="ps_out")
                    for j in range(2):
                        nc.tensor.matmul(
                            ps,
                            woutT_sb[:, 2 * j:2 * j + 2, t * 128:(t + 1) * 128],
                            ao[:, 2 * j:2 * j + 2, n * 512:(n + 1) * 512],
                            start=(j == 0), stop=(j == 1), perf_mode=DR)
                    o_sb = outp.tile([128, 512], bf16, tag="o", name="o_sb")
                    nc.vector.tensor_add(out=o_sb, in0=ps, in1=xr[:, t, n * 512:(n + 1) * 512])
                    # late slices store over the sync ring (idle once loads are
                    # done) so the final drain splits across two DMA rings
                    eng = nc.sync if s >= S - 2 else nc.gpsimd
                    eng.dma_start(
                        out=out_d.ap()[s, t * 128:(t + 1) * 128, n * 512:(n + 1) * 512],
                        in_=o_sb)

            # ---- A(0): projections for slice 0 up front ----
            # copies alternate Act/DVE: in this phase the chunk rate is
            # copy-bound, and both engines are otherwise idle
            prefetch(1)
            a_alloc(0)
            # token-half-major order: B(0)'s attention pairs 0-1 read only the
            # n=0 half of qk(0), so they can start while the n=1 half and V^T
            # chunks are still in flight (region-tracked)
            for t in range(8):
                a_qk_half(0, t, 0, dve=(t % 2 == 1))
            for t in range(8):
                a_qk_half(0, t, 1, dve=(t % 2 == 1))
            for g in range(8):
                a_vt_chunk(0, g, dve=(g % 2 == 1))

            # ---- main loop: attention(s) interleaved with projections(s+1) ----
            # Per-slice PSUM tiles, padded so the dim1 stride is one 2KB bank
            # (HW rule: concurrent quadrant matmuls sharing a PE column-group
            # must accumulate in different banks). Generations g alternate
            # between the two 128-column halves of the same banks, so
            # scores(g+1)/AV(g+1) never serialize behind exp(g)/copy(g).
            for s in range(S):
                prefetch(s + 2)
                if s + 1 < S:
                    a_alloc(s + 1)
                qk, vt = qkt[s], vtt[s]
                ao = aop.tile([128, 4, NTOK], f8, tag="ao", name=f"ao_{s}")
                s_ps = pss.tile([128, 2, 512], f32, tag="s", name=f"s_ps_{s}")
                av_ps = psav.tile([128, 4, 512], f32, tag="av", name=f"av_ps_{s}")
                qk_next = s + 1 if s + 1 < S else None
                vt_next = qk_next
                t_tiles = {}

                def scores_pair(k):
                    # groups (2k, 2k+1) -> score banks half (k%2)*256, layout
                    # s_ps[(w',i), par, base + u*128 + (h2,j)], u = group-in-pair
                    base = (k % 2) * 256
                    for u in range(2):
                        g = 2 * k + u
                        for q in range(4):
                            for wq in range(4):
                                for par in range(2):
                                    n = 2 * q + par
                                    bp = 64 * par
                                    toff = (4 * g + wq) * 32
                                    qa = qk[bp:bp + 64, n // 2, toff:toff + 32]
                                    ka = qk[bp:bp + 64, 4 + n // 2, toff:toff + 32]
                                    co = base + u * 128 + q * 32
                                    nc.tensor.matmul(
                                        s_ps[wq * 32:wq * 32 + 32, par, co:co + 32],
                                        qa, ka, start=True, stop=True,
                                        tile_position=(bp, wq * 32))

                def softmax_pair(k):
                    # softmax (no max-sub; logits are small by construction)
                    base = (k % 2) * 256
                    p_sb = pp.tile([128, 2, 256], bf16, tag="p", name="p_sb")
                    t_sb = ttp.tile([128, 2, 256], bf16, tag="t", name="t_sb")
                    sums = smp.tile([128, 16], f32, tag="sums", name="sums")
                    nc.scalar.activation(
                        out=p_sb, in_=s_ps[:, :, base:base + 256],
                        func=Act.Exp, scale=float(HD) ** -0.5 / 2)
                    pv = p_sb.rearrange("p a (h j) -> p (a h) j", h=8)
                    nc.vector.reduce_sum(out=sums, in_=pv, axis=mybir.AxisListType.X)
                    nc.vector.reciprocal(out=sums, in_=sums)
                    nc.gpsimd.tensor_mul(
                        out=pv, in0=pv,
                        in1=sums.unsqueeze(2).broadcast_to([128, 16, 32]))
                    nc.vector.transpose(
                        out=t_sb.rearrange("p a f -> p (a f)"),
                        in_=p_sb.rearrange("p a f -> p (a f)"))
                    t_tiles[k] = t_sb

                def av_pair(k):
                    base = (k % 2) * 256
                    tt = t_tiles[k]
                    for u in range(2):
                        g = 2 * k + u
                        for q in range(4):
                            for wq in range(4):
                                for par in range(2):
                                    n = 2 * q + par
                                    lhsT = vt[wq * 32:wq * 32 + 32, g, n * 64:n * 64 + 64]
                                    rhs = tt[wq * 32:wq * 32 + 32, par, u * 128 + q * 32:u * 128 + q * 32 + 32]
                                    co = base + u * 128 + q * 32
                                    nc.tensor.matmul(
                                        av_ps[par * 64:par * 64 + 64, wq, co:co + 32],
                                        lhsT, rhs, start=True, stop=True,
                                        tile_position=(wq * 32, par * 64))

                def av_copy_pair(k):
                    base = (k % 2) * 256
                    t_tiles.pop(k)
                    for u in range(2):
                        g = 2 * k + u
                        dst = ao[:, :, g * 128:g * 128 + 128].rearrange(
                            "p q (w i) -> p q w i", i=32)
                        srcv = av_ps[:, :, base + u * 128:base + u * 128 + 128].rearrange(
                            "p w (q i) -> p q w i", i=32)
                        if s == S - 1 or k == 3:
                            # drain iterations run no exp, so Act is idle there
                            # while DVE's chain otherwise delays ao -> out-proj
                            nc.scalar.activation(out=dst, in_=srcv, func=Act.Copy)
                        else:
                            nc.vector.tensor_copy(out=dst, in_=srcv)

                # QK(s+1) chunks front-loaded (B(s+1)'s first scores read
                # qk(s+1), so its last copies must retire well before B(s)
                # ends); VT(s+1) fills pgi 3-4, interleaved with the out
                # projection so psmm allocations are never back-to-back.
                qk_sched = [[(0, 0), (1, 0), (2, 0), (3, 0), (4, 0)],
                            [(5, 0), (6, 0), (7, 0), (0, 1), (1, 1)],
                            [(2, 1), (3, 1), (4, 1), (5, 1), (6, 1), (7, 1)],
                            [], []]

                for pgi in range(5):
                    qks = qk_sched[pgi] if qk_next is not None else []
                    vt_on = vt_next is not None
                    if pgi < 4:
                        scores_pair(pgi)
                        softmax_pair(pgi)
                    for t, n in qks[:2]:
                        a_qk_half(qk_next, t, n)
                    for t, n in qks[2:4]:
                        a_qk_half(qk_next, t, n)
                    if pgi == 3 and vt_on:
                        a_vt_chunk(vt_next, 0)
                        a_vt_chunk(vt_next, 1)
                    if pgi == 4 and vt_on:
                        a_vt_chunk(vt_next, 4)
                        a_vt_chunk(vt_next, 5)
                    if pgi >= 1:
                        av_pair(pgi - 1)
                    for t, n in qks[4:]:
                        a_qk_half(qk_next, t, n)
                    if pgi == 3 and vt_on:
                        a_vt_chunk(vt_next, 2)
                        a_vt_chunk(vt_next, 3)
                    if pgi >= 1:
                        av_copy_pair(pgi - 1)
                    if pgi == 2:
                        out_half(s, 0, ao, 0, 2)
                    if pgi == 3:
                        out_half(s, 0, ao, 2, 4)
                    if pgi == 4:
                        out_half(s, 1, ao, 0, 2)
                        if vt_on:
                            a_vt_chunk(vt_next, 6)
                            a_vt_chunk(vt_next, 7)
                        out_half(s, 1, ao, 2, 4)

    nc.compile()
    return nc


_NC = None


def kernel(x, w_qkv, b_qkv, w_out, b_out):
    global _NC, LAST_RESULTS
    from concourse import bass_utils

    f8 = ml_dtypes.float8_e4m3
    bf = ml_dtypes.bfloat16
    x = np.asarray(x, dtype=np.float32)
    w_qkv = np.asarray(w_qkv, dtype=np.float32)
    b_qkv = np.asarray(b_qkv, dtype=np.float32)
    w_out = np.asarray(w_out, dtype=np.float32)
    b_out = np.asarray(b_out, dtype=np.float32)

    wqkT = np.ascontiguousarray(w_qkv[:2 * C].T * WS).astype(f8)   # [C, 2C] x16
    wvT = np.ascontiguousarray(w_qkv[2 * C:].T * WS).astype(f8)    # [C, C] x16
    woutT = np.ascontiguousarray(w_out.T * WS).astype(f8)          # [C, C] x16
    bqk = np.ascontiguousarray(b_qkv[:2 * C]).astype(np.float32)
    # b_v commutes through attention (rows of softmax sum to 1) -> fold into b_out
    bout_eff = (b_out + w_out @ b_qkv[2 * C:]).astype(np.float32)

    if _NC is None:
        _NC = _build()

    in_maps = []
    for cid in range(NCORES):
        xs8 = np.empty((S, C, NTOK), dtype=f8)
        xrs = np.empty((S, C, NTOK), dtype=bf)
        for i in range(S):
            gs = cid * S + i
            b, h = gs // H, gs % H
            xw = x[b, :, :, h, :].transpose(0, 2, 1)  # [C, W, D] w-major tokens
            xs8[i] = xw.reshape(C, NTOK).astype(f8)
            xrs[i] = ((xw + bout_eff[:, None, None]) * WS).reshape(C, NTOK).astype(bf)
        in_maps.append(dict(xs8=xs8, xres=xrs, wqkT=wqkT, wvT=wvT,
                            woutT=woutT, bqk=bqk))

    res = bass_utils.run_bass_kernel_spmd(
        _NC, in_maps, core_ids=list(range(NCORES)),
        trace=bool(os.environ.get("BASS_TRACE")))
    LAST_RESULTS = res

    out = np.empty((B, C, D, H, W), dtype=np.float32)
    for cid in range(NCORES):
        o = res.results[cid]["out"]  # [S, C, 1024] bf16, w-major tokens, x16
        for i in range(S):
            gs = cid * S + i
            b, h = gs // H, gs % H
            out[b, :, :, h, :] = (
                o[i].astype(np.float32) * (1.0 / WS)
            ).reshape(C, W, D).transpose(0, 2, 1)
    return out
